# revision 19
# baseline (speedup 1.0000x reference)
"""MoE (AriaExperts) Trainium2 kernel — expert parallelism across 8 NeuronCores.

Strategy:
  - Host: top-2 routing + softmax over [2048, 8] logits (tiny), build the
    per-expert token batches (the "all-to-all" is realized at input
    distribution time), and the weighted scatter-add combine at the end.
  - Device (SPMD, 1 expert per core): dense GEMM chain in transposed
    activation layout so both matmuls consume the expert weights directly
    as the stationary (lhsT) operand with zero on-device transposes:
        H^T  = W1^T-tiles @ X^T      [2*INTER, C]
        actT = silu(projT) * gateT   [INTER, C]
        outT = W2-tiles   @ actT     [HIDDEN, C]
    bf16 matmuls with f32 PSUM accumulation (1 cycle/row vs 4 for f32).
  - Each core processes C = (max expert token count, padded) columns split
    into chunks of CH <= 512 (PSUM bank limit), processed in blocks of two
    chunks; shapes are chosen at runtime from the actual routing and the
    graph is compiled per-shape and cached.
"""

import time

import numpy as np
import ml_dtypes

import concourse.bass as bass
import concourse.bacc as bacc
import concourse.mybir as mybir
import concourse.tile as tile
from concourse.bass_utils import run_bass_kernel_spmd

NUM_TOKENS = 2048
HIDDEN = 1024
INTER = 2048
NUM_EXPERTS = 8
TOPK = 2
NCORES = 8
P = 128
KT1 = HIDDEN // P         # 8  k-tiles (FC1 contraction)
MT1 = 2 * INTER // P      # 32 m-tiles (FC1 output rows = proj+gate)
MT1H = INTER // P         # 16 proj/gate pair count
KT2 = INTER // P          # 16 k-tiles (FC2 contraction)
MT2 = HIDDEN // P         # 8  m-tiles (FC2 output rows)

BF16 = mybir.dt.bfloat16
F32 = mybir.dt.float32
np_bf16 = ml_dtypes.bfloat16

# [0, 16, 1, 17, ...] — interleave proj/gate m-tiles into adjacent pairs
_W1_ORDER = np.arange(MT1).reshape(2, MT1H).T.reshape(-1)

_graph_cache: dict = {}


def _build(NCH: int, CH: int) -> bass.Bass:
    """Per-core Bass graph for capacity C_pad = NCH * CH (CH <= 512).

    Columns are processed in blocks of up to two chunks; each PSUM tile is
    [P, 2, 512] f32 = 2 banks, so 4 pool slots fill all 8 banks.
    """
    C_pad = NCH * CH
    nc = bacc.Bacc("TRN2", target_bir_lowering=False, debug=False)

    xt_d = nc.declare_dram_parameter("xt", [P, KT1, NCH, CH], BF16, isOutput=False)
    w1_d = nc.declare_dram_parameter("w1", [P, MT1, KT1, P], BF16, isOutput=False)
    w2_d = nc.declare_dram_parameter("w2", [P, MT2, KT2, P], BF16, isOutput=False)
    # bf16 output: halves the output DMA on the kernel tail; the host-side
    # combine upcasts to f32 (adds ~0.2% rounding on top of the ~0.4% bf16
    # matmul error — well within the 2e-2 gate).
    out_d = nc.declare_dram_parameter("out", [MT2, NCH, P, CH], BF16, isOutput=True)

    # Blocks of up to 2 chunks each
    blocks = []
    ch0 = 0
    while ch0 < NCH:
        blocks.append((ch0, min(2, NCH - ch0)))
        ch0 += min(2, NCH - ch0)

    # w1 DMA chunk sizes in proj/gate PAIRS (host layout interleaves
    # proj mt / gate mt+16 adjacently so pair mt only needs chunk ~mt/2):
    # fine-grained at the front so the first pairs start ASAP.
    w1_chunks = [1, 1, 2, 2, 2, 2, 2, 2, 2]
    assert sum(w1_chunks) == MT1H

    with tile.TileContext(nc) as tc:
        with (
            tc.tile_pool(name="weights", bufs=1) as wpool,
            tc.tile_pool(name="xin", bufs=1) as xpool,
            tc.tile_pool(name="actp", bufs=2) as apool,
            tc.tile_pool(name="tmp", bufs=2) as tpool,
            tc.tile_pool(name="osb", bufs=2) as opool,
            tc.tile_pool(name="psum", bufs=4, space="PSUM") as pspool,
        ):
            xt = xpool.tile([P, KT1, NCH, CH], BF16, tag="xt")
            w1 = wpool.tile([P, MT1H, 2, KT1, P], BF16, tag="w1")
            w2 = wpool.tile([P, MT2, KT2, P], BF16, tag="w2")
            dummy = xpool.tile([P, 640], BF16, tag="dummy")

            # PE warmup: ~20 back-to-back matmuls on a memset tile so the
            # HAM clock-gate reaches K=8/8 while input DMAs are in flight
            # (otherwise the first ~15us of real matmuls run at 1.2 GHz).
            nc.gpsimd.memset(dummy[:], 0.0)
            warm_ps = pspool.tile([P, 2, 512], F32, tag="ps", name="warmps")
            for _ in range(20):
                nc.tensor.matmul(
                    warm_ps[:, 0, :], dummy[:, :128], dummy[:, 128:640],
                    start=True, stop=True,
                )

            # Input DMAs on BOTH HWDGE rings (SP + ACT) — triggers cost
            # ~650ns each and serialize per ring; the two rings share HBM
            # bandwidth, so the first-needed bytes are split across both.
            hk = KT1 // 2
            nc.sync.dma_start(out=xt[:, :hk], in_=xt_d[:, :hk])
            nc.scalar.dma_start(out=xt[:, hk:], in_=xt_d[:, hk:])
            pr0 = 0
            for ci, cw in enumerate(w1_chunks):
                eng = nc.sync if ci % 2 == 0 else nc.scalar
                eng.dma_start(
                    out=w1[:, pr0 : pr0 + cw],
                    in_=w1_d[:, 2 * pr0 : 2 * (pr0 + cw)],
                )
                pr0 += cw
            for g in range(MT2 // 4):
                eng = nc.sync if g % 2 == 0 else nc.scalar
                eng.dma_start(
                    out=w2[:, g * 4 : (g + 1) * 4], in_=w2_d[:, g * 4 : (g + 1) * 4]
                )

            for bi, (ch0, bch) in enumerate(blocks):
                # ---- FC1 (proj/gate pair per iteration) + SwiGLU ----
                act = apool.tile([P, KT2, 2, CH], BF16, tag="act", name=f"act{bi}")
                for mt in range(MT1H):
                    ps_p = pspool.tile(
                        [P, 2, 512], F32, tag="ps", name=f"psp{bi}_{mt}"
                    )
                    ps_g = pspool.tile(
                        [P, 2, 512], F32, tag="ps", name=f"psg{bi}_{mt}"
                    )
                    for ps, pg in ((ps_p, 0), (ps_g, 1)):
                        for kt in range(KT1):
                            for j in range(bch):
                                nc.tensor.matmul(
                                    ps[:, j, :CH],
                                    w1[:, mt, pg, kt, :],
                                    xt[:, kt, ch0 + j, :],
                                    start=(kt == 0),
                                    stop=(kt == KT1 - 1),
                                )
                    # SwiGLU: silu on ACT, multiply on DVE (Bacc's
                    # generate_event_semaphores legalizes multi-wait insts).
                    tmp = tpool.tile([P, 2, CH], F32, tag="tmp", name=f"tmp{bi}_{mt}")
                    for j in range(bch):
                        nc.scalar.activation(
                            tmp[:, j], ps_p[:, j, :CH],
                            mybir.ActivationFunctionType.Silu,
                        )
                        nc.vector.tensor_mul(act[:, mt, j], tmp[:, j], ps_g[:, j, :CH])

                # ---- FC2 ----
                for m2 in range(MT2):
                    ps_o = pspool.tile(
                        [P, 2, 512], F32, tag="ps", name=f"pso{bi}_{m2}"
                    )
                    for kt2 in range(KT2):
                        for j in range(bch):
                            nc.tensor.matmul(
                                ps_o[:, j, :CH],
                                w2[:, m2, kt2, :],
                                act[:, kt2, j, :],
                                start=(kt2 == 0),
                                stop=(kt2 == KT2 - 1),
                            )
                    o_sb = opool.tile([P, 2, CH], BF16, tag="o", name=f"osb{bi}_{m2}")
                    for j in range(bch):
                        nc.scalar.copy(o_sb[:, j], ps_o[:, j, :CH])
                        nc.sync.dma_start(
                            out=out_d[m2, ch0 + j], in_=o_sb[:, j]
                        )

    nc.compile()
    return nc


def _get_graph(NCH: int, CH: int) -> bass.Bass:
    key = (NCH, CH)
    if key not in _graph_cache:
        _graph_cache[key] = _build(NCH, CH)
    return _graph_cache[key]


def _route(router_logits: np.ndarray):
    """Top-2 + softmax, exactly matching jax.lax.top_k tie-breaking."""
    idx = np.argsort(-router_logits, axis=-1, kind="stable")[:, :TOPK]
    tl = np.take_along_axis(router_logits, idx, axis=-1)
    ex = np.exp(tl - tl.max(-1, keepdims=True))
    sc = (ex / ex.sum(-1, keepdims=True)).astype(np.float32)
    return idx, sc


def run(hidden_states, router_logits, w1, w2, trace=False, trace_kwargs=None):
    hs = np.asarray(hidden_states, dtype=np.float32)
    rl = np.asarray(router_logits, dtype=np.float32)
    w1 = np.asarray(w1, dtype=np.float32)
    w2 = np.asarray(w2, dtype=np.float32)
    N, D = hs.shape

    idx, sc = _route(rl)

    tok_lists = []
    for e in range(NUM_EXPERTS):
        toks, slots = np.nonzero(idx == e)
        tok_lists.append((toks, slots))
    cmax = max(len(t) for t, _ in tok_lists)

    NCH = max(1, -(-cmax // 512))
    CH = -(-cmax // (NCH * 2)) * 2  # chunk width, multiple of 2
    C_pad = CH * NCH

    in_maps = []
    for e in range(NUM_EXPERTS):
        toks, _ = tok_lists[e]
        x = np.zeros((C_pad, D), np.float32)
        x[: len(toks)] = hs[toks]
        xt = x.T.reshape(KT1, P, NCH, CH).transpose(1, 0, 2, 3).astype(np_bf16)
        # [p, mt, kt, m] with the mt axis pair-interleaved: proj tile mt and
        # gate tile mt+MT1H land adjacently so pair mt needs one DMA chunk.
        w1e = w1[e].reshape(KT1, P, MT1, P).transpose(1, 2, 0, 3)[:, _W1_ORDER]
        w1e = w1e.astype(np_bf16)
        w2e = w2[e].reshape(KT2, P, MT2, P).transpose(1, 2, 0, 3).astype(np_bf16)
        in_maps.append({"xt": xt, "w1": w1e, "w2": w2e})

    nc = _get_graph(NCH, CH)

    res = None
    for attempt in range(4):
        try:
            res = run_bass_kernel_spmd(
                nc,
                in_maps,
                core_ids=list(range(NCORES)),
                trace=trace,
                **(trace_kwargs or {}),
            )
            break
        except Exception:
            if attempt == 3:
                raise
            time.sleep(15 * (attempt + 1))

    out = np.zeros((N, D), np.float32)
    for e in range(NUM_EXPERTS):
        toks, slots = tok_lists[e]
        oT = np.asarray(res.results[e]["out"]).astype(np.float32)
        oT = oT.transpose(0, 2, 1, 3).reshape(HIDDEN, C_pad)
        out[toks] += sc[toks, slots][:, None] * oT[:, : len(toks)].T
    return out, res


def kernel(hidden_states, router_logits, w1, w2):
    out, _ = run(hidden_states, router_logits, w1, w2)
    return out


# revision 20
# speedup vs baseline: 1.0074x; 1.0074x over previous
"""MoE (AriaExperts) Trainium2 kernel — expert parallelism across 8 NeuronCores.

Strategy:
  - Host: top-2 routing + softmax over [2048, 8] logits (tiny), build the
    per-expert token batches (the "all-to-all" is realized at input
    distribution time), and the weighted scatter-add combine at the end.
  - Device (SPMD, 1 expert per core): dense GEMM chain in transposed
    activation layout so both matmuls consume the expert weights directly
    as the stationary (lhsT) operand with zero on-device transposes:
        H^T  = W1^T-tiles @ X^T      [2*INTER, C]
        actT = silu(projT) * gateT   [INTER, C]
        outT = W2-tiles   @ actT     [HIDDEN, C]
    bf16 matmuls with f32 PSUM accumulation (1 cycle/row vs 4 for f32).
  - Each core processes C = (max expert token count, padded) columns split
    into chunks of CH <= 512 (PSUM bank limit), processed in blocks of two
    chunks; shapes are chosen at runtime from the actual routing and the
    graph is compiled per-shape and cached.
"""

import time

import numpy as np
import ml_dtypes

import concourse.bass as bass
import concourse.bacc as bacc
import concourse.mybir as mybir
import concourse.tile as tile
from concourse.bass_utils import run_bass_kernel_spmd

NUM_TOKENS = 2048
HIDDEN = 1024
INTER = 2048
NUM_EXPERTS = 8
TOPK = 2
NCORES = 8
P = 128
KT1 = HIDDEN // P         # 8  k-tiles (FC1 contraction)
MT1 = 2 * INTER // P      # 32 m-tiles (FC1 output rows = proj+gate)
MT1H = INTER // P         # 16 proj/gate pair count
KT2 = INTER // P          # 16 k-tiles (FC2 contraction)
MT2 = HIDDEN // P         # 8  m-tiles (FC2 output rows)

BF16 = mybir.dt.bfloat16
F32 = mybir.dt.float32
np_bf16 = ml_dtypes.bfloat16

# [0, 16, 1, 17, ...] — interleave proj/gate m-tiles into adjacent pairs
_W1_ORDER = np.arange(MT1).reshape(2, MT1H).T.reshape(-1)

_graph_cache: dict = {}


def _build(NCH: int, CH: int) -> bass.Bass:
    """Per-core Bass graph for capacity C_pad = NCH * CH (CH <= 512).

    Columns are processed in blocks of up to two chunks; each PSUM tile is
    [P, 2, 512] f32 = 2 banks, so 4 pool slots fill all 8 banks.
    """
    C_pad = NCH * CH
    nc = bacc.Bacc("TRN2", target_bir_lowering=False, debug=False)

    xt_d = nc.declare_dram_parameter("xt", [P, KT1, NCH, CH], BF16, isOutput=False)
    w1_d = nc.declare_dram_parameter("w1", [P, MT1, KT1, P], BF16, isOutput=False)
    w2_d = nc.declare_dram_parameter("w2", [P, MT2, KT2, P], BF16, isOutput=False)
    # bf16 output: halves the output DMA on the kernel tail; the host-side
    # combine upcasts to f32 (adds ~0.2% rounding on top of the ~0.4% bf16
    # matmul error — well within the 2e-2 gate).
    out_d = nc.declare_dram_parameter("out", [MT2, NCH, P, CH], BF16, isOutput=True)

    # Blocks of up to 2 chunks each
    blocks = []
    ch0 = 0
    while ch0 < NCH:
        blocks.append((ch0, min(2, NCH - ch0)))
        ch0 += min(2, NCH - ch0)

    # w1 DMA chunk sizes in proj/gate PAIRS (host layout interleaves
    # proj mt / gate mt+16 adjacently so pair mt only needs chunk ~mt/2):
    # fine-grained at the front so the first pairs start ASAP.
    w1_chunks = [1, 1, 2, 2, 2, 2, 2, 2, 2]
    assert sum(w1_chunks) == MT1H

    with tile.TileContext(nc) as tc:
        with (
            tc.tile_pool(name="weights", bufs=1) as wpool,
            tc.tile_pool(name="xin", bufs=1) as xpool,
            tc.tile_pool(name="actp", bufs=2) as apool,
            tc.tile_pool(name="tmp", bufs=2) as tpool,
            tc.tile_pool(name="osb", bufs=2) as opool,
            tc.tile_pool(name="psum", bufs=4, space="PSUM") as pspool,
        ):
            xt = xpool.tile([P, KT1, NCH, CH], BF16, tag="xt")
            w1 = wpool.tile([P, MT1H, 2, KT1, P], BF16, tag="w1")
            w2 = wpool.tile([P, MT2, KT2, P], BF16, tag="w2")
            dummy = xpool.tile([P, 640], BF16, tag="dummy")

            # PE warmup: ~20 back-to-back matmuls on a memset tile so the
            # HAM clock-gate reaches K=8/8 while input DMAs are in flight
            # (otherwise the first ~15us of real matmuls run at 1.2 GHz).
            nc.gpsimd.memset(dummy[:], 0.0)
            warm_ps = pspool.tile([P, 2, 512], F32, tag="ps", name="warmps")
            for _ in range(20):
                nc.tensor.matmul(
                    warm_ps[:, 0, :], dummy[:, :128], dummy[:, 128:640],
                    start=True, stop=True,
                )

            # Input DMAs on BOTH HWDGE rings (SP + ACT) — triggers cost
            # ~650ns each and serialize per ring; the two rings share HBM
            # bandwidth, so the first-needed bytes are split across both.
            hk = KT1 // 2
            nc.sync.dma_start(out=xt[:, :hk], in_=xt_d[:, :hk])
            nc.scalar.dma_start(out=xt[:, hk:], in_=xt_d[:, hk:])
            pr0 = 0
            for ci, cw in enumerate(w1_chunks):
                eng = nc.sync if ci % 2 == 0 else nc.scalar
                eng.dma_start(
                    out=w1[:, pr0 : pr0 + cw],
                    in_=w1_d[:, 2 * pr0 : 2 * (pr0 + cw)],
                )
                pr0 += cw
            for g in range(MT2 // 4):
                eng = nc.sync if g % 2 == 0 else nc.scalar
                eng.dma_start(
                    out=w2[:, g * 4 : (g + 1) * 4], in_=w2_d[:, g * 4 : (g + 1) * 4]
                )

            for bi, (ch0, bch) in enumerate(blocks):
                # ---- FC1 (proj/gate pair per iteration) + SwiGLU ----
                act = apool.tile([P, KT2, 2, CH], BF16, tag="act", name=f"act{bi}")
                for mt in range(MT1H):
                    ps_p = pspool.tile(
                        [P, 2, 512], F32, tag="ps", name=f"psp{bi}_{mt}"
                    )
                    ps_g = pspool.tile(
                        [P, 2, 512], F32, tag="ps", name=f"psg{bi}_{mt}"
                    )
                    for ps, pg in ((ps_p, 0), (ps_g, 1)):
                        for kt in range(KT1):
                            for j in range(bch):
                                nc.tensor.matmul(
                                    ps[:, j, :CH],
                                    w1[:, mt, pg, kt, :],
                                    xt[:, kt, ch0 + j, :],
                                    start=(kt == 0),
                                    stop=(kt == KT1 - 1),
                                )
                    # SwiGLU: silu on ACT, multiply on DVE (Bacc's
                    # generate_event_semaphores legalizes multi-wait insts).
                    tmp = tpool.tile([P, 2, CH], F32, tag="tmp", name=f"tmp{bi}_{mt}")
                    for j in range(bch):
                        nc.scalar.activation(
                            tmp[:, j], ps_p[:, j, :CH],
                            mybir.ActivationFunctionType.Silu,
                        )
                        nc.vector.tensor_mul(act[:, mt, j], tmp[:, j], ps_g[:, j, :CH])

                # ---- FC2 ----
                for m2 in range(MT2):
                    ps_o = pspool.tile(
                        [P, 2, 512], F32, tag="ps", name=f"pso{bi}_{m2}"
                    )
                    for kt2 in range(KT2):
                        for j in range(bch):
                            nc.tensor.matmul(
                                ps_o[:, j, :CH],
                                w2[:, m2, kt2, :],
                                act[:, kt2, j, :],
                                start=(kt2 == 0),
                                stop=(kt2 == KT2 - 1),
                            )
                    o_sb = opool.tile([P, 2, CH], BF16, tag="o", name=f"osb{bi}_{m2}")
                    nc.scalar.copy(o_sb[:, :bch], ps_o[:, :bch, :CH])
                    eng = nc.sync if m2 % 2 == 0 else nc.scalar
                    eng.dma_start(
                        out=out_d[m2, ch0 : ch0 + bch].rearrange("b p c -> p b c"),
                        in_=o_sb[:, :bch],
                    )

    nc.compile()
    return nc


def _get_graph(NCH: int, CH: int) -> bass.Bass:
    key = (NCH, CH)
    if key not in _graph_cache:
        _graph_cache[key] = _build(NCH, CH)
    return _graph_cache[key]


def _route(router_logits: np.ndarray):
    """Top-2 + softmax, exactly matching jax.lax.top_k tie-breaking."""
    idx = np.argsort(-router_logits, axis=-1, kind="stable")[:, :TOPK]
    tl = np.take_along_axis(router_logits, idx, axis=-1)
    ex = np.exp(tl - tl.max(-1, keepdims=True))
    sc = (ex / ex.sum(-1, keepdims=True)).astype(np.float32)
    return idx, sc


def run(hidden_states, router_logits, w1, w2, trace=False, trace_kwargs=None):
    hs = np.asarray(hidden_states, dtype=np.float32)
    rl = np.asarray(router_logits, dtype=np.float32)
    w1 = np.asarray(w1, dtype=np.float32)
    w2 = np.asarray(w2, dtype=np.float32)
    N, D = hs.shape

    idx, sc = _route(rl)

    tok_lists = []
    for e in range(NUM_EXPERTS):
        toks, slots = np.nonzero(idx == e)
        tok_lists.append((toks, slots))
    cmax = max(len(t) for t, _ in tok_lists)

    NCH = max(1, -(-cmax // 512))
    CH = -(-cmax // (NCH * 2)) * 2  # chunk width, multiple of 2
    C_pad = CH * NCH

    in_maps = []
    for e in range(NUM_EXPERTS):
        toks, _ = tok_lists[e]
        x = np.zeros((C_pad, D), np.float32)
        x[: len(toks)] = hs[toks]
        xt = x.T.reshape(KT1, P, NCH, CH).transpose(1, 0, 2, 3).astype(np_bf16)
        # [p, mt, kt, m] with the mt axis pair-interleaved: proj tile mt and
        # gate tile mt+MT1H land adjacently so pair mt needs one DMA chunk.
        w1e = w1[e].reshape(KT1, P, MT1, P).transpose(1, 2, 0, 3)[:, _W1_ORDER]
        w1e = w1e.astype(np_bf16)
        w2e = w2[e].reshape(KT2, P, MT2, P).transpose(1, 2, 0, 3).astype(np_bf16)
        in_maps.append({"xt": xt, "w1": w1e, "w2": w2e})

    nc = _get_graph(NCH, CH)

    res = None
    for attempt in range(4):
        try:
            res = run_bass_kernel_spmd(
                nc,
                in_maps,
                core_ids=list(range(NCORES)),
                trace=trace,
                **(trace_kwargs or {}),
            )
            break
        except Exception:
            if attempt == 3:
                raise
            time.sleep(15 * (attempt + 1))

    out = np.zeros((N, D), np.float32)
    for e in range(NUM_EXPERTS):
        toks, slots = tok_lists[e]
        oT = np.asarray(res.results[e]["out"]).astype(np.float32)
        oT = oT.transpose(0, 2, 1, 3).reshape(HIDDEN, C_pad)
        out[toks] += sc[toks, slots][:, None] * oT[:, : len(toks)].T
    return out, res


def kernel(hidden_states, router_logits, w1, w2):
    out, _ = run(hidden_states, router_logits, w1, w2)
    return out


# revision 22
# speedup vs baseline: 1.0101x; 1.0027x over previous
"""MoE (AriaExperts) Trainium2 kernel — expert parallelism across 8 NeuronCores.

Strategy:
  - Host: top-2 routing + softmax over [2048, 8] logits (tiny), build the
    per-expert token batches (the "all-to-all" is realized at input
    distribution time), and the weighted scatter-add combine at the end.
  - Device (SPMD, 1 expert per core): dense GEMM chain in transposed
    activation layout so both matmuls consume the expert weights directly
    as the stationary (lhsT) operand with zero on-device transposes:
        H^T  = W1^T-tiles @ X^T      [2*INTER, C]
        actT = silu(projT) * gateT   [INTER, C]
        outT = W2-tiles   @ actT     [HIDDEN, C]
    bf16 matmuls with f32 PSUM accumulation (1 cycle/row vs 4 for f32).
  - Each core processes C = (max expert token count, padded) columns split
    into chunks of CH <= 512 (PSUM bank limit), processed in blocks of two
    chunks; shapes are chosen at runtime from the actual routing and the
    graph is compiled per-shape and cached.
"""

import time

import numpy as np
import ml_dtypes

import concourse.bass as bass
import concourse.bacc as bacc
import concourse.mybir as mybir
import concourse.tile as tile
from concourse.bass_utils import run_bass_kernel_spmd

NUM_TOKENS = 2048
HIDDEN = 1024
INTER = 2048
NUM_EXPERTS = 8
TOPK = 2
NCORES = 8
P = 128
KT1 = HIDDEN // P         # 8  k-tiles (FC1 contraction)
MT1 = 2 * INTER // P      # 32 m-tiles (FC1 output rows = proj+gate)
MT1H = INTER // P         # 16 proj/gate pair count
KT2 = INTER // P          # 16 k-tiles (FC2 contraction)
MT2 = HIDDEN // P         # 8  m-tiles (FC2 output rows)

BF16 = mybir.dt.bfloat16
F32 = mybir.dt.float32
np_bf16 = ml_dtypes.bfloat16

# [0, 16, 1, 17, ...] — interleave proj/gate m-tiles into adjacent pairs
_W1_ORDER = np.arange(MT1).reshape(2, MT1H).T.reshape(-1)

_graph_cache: dict = {}


def _build(NCH: int, CH: int) -> bass.Bass:
    """Per-core Bass graph for capacity C_pad = NCH * CH (CH <= 512).

    Columns are processed in blocks of up to two chunks; each PSUM tile is
    [P, 2, 512] f32 = 2 banks, so 4 pool slots fill all 8 banks.
    """
    C_pad = NCH * CH
    nc = bacc.Bacc("TRN2", target_bir_lowering=False, debug=False)

    xt_d = nc.declare_dram_parameter("xt", [P, KT1, NCH, CH], BF16, isOutput=False)
    w1_d = nc.declare_dram_parameter("w1", [P, MT1, KT1, P], BF16, isOutput=False)
    w2_d = nc.declare_dram_parameter("w2", [P, MT2, KT2, P], BF16, isOutput=False)
    # bf16 output: halves the output DMA on the kernel tail; the host-side
    # combine upcasts to f32 (adds ~0.2% rounding on top of the ~0.4% bf16
    # matmul error — well within the 2e-2 gate).
    out_d = nc.declare_dram_parameter("out", [MT2, NCH, P, CH], BF16, isOutput=True)

    # Blocks of up to 2 chunks each
    blocks = []
    ch0 = 0
    while ch0 < NCH:
        blocks.append((ch0, min(2, NCH - ch0)))
        ch0 += min(2, NCH - ch0)

    # w1 DMA chunk sizes in proj/gate PAIRS (host layout interleaves
    # proj mt / gate mt+16 adjacently so pair mt only needs chunk ~mt/2):
    # fine-grained at the front so the first pairs start ASAP.
    w1_chunks = [1, 1, 2, 2, 2, 2, 2, 2, 2]
    assert sum(w1_chunks) == MT1H

    with tile.TileContext(nc) as tc:
        with (
            tc.tile_pool(name="weights", bufs=1) as wpool,
            tc.tile_pool(name="xin", bufs=1) as xpool,
            tc.tile_pool(name="actp", bufs=1) as apool,
            tc.tile_pool(name="tmp", bufs=2) as tpool,
            tc.tile_pool(name="osb", bufs=2) as opool,
            tc.tile_pool(name="psum", bufs=4, space="PSUM") as pspool,
        ):
            # Per-block xt tiles so SBUF stays bounded for any routing skew
            # (worst case one expert owns all 4096 token-slots -> NCH=8).
            xts = [
                xpool.tile([P, KT1, 2, CH], BF16, tag="xt", name=f"xt{bi}")
                for bi in range(len(blocks))
            ]
            w1 = wpool.tile([P, MT1H, 2, KT1, P], BF16, tag="w1")
            w2 = wpool.tile([P, MT2, KT2, P], BF16, tag="w2")
            dummy = xpool.tile([P, 640], BF16, tag="dummy")

            # PE warmup: ~20 back-to-back matmuls on a memset tile so the
            # HAM clock-gate reaches K=8/8 while input DMAs are in flight
            # (otherwise the first ~15us of real matmuls run at 1.2 GHz).
            nc.gpsimd.memset(dummy[:], 0.0)
            warm_ps = pspool.tile([P, 2, 512], F32, tag="ps", name="warmps")
            for _ in range(20):
                nc.tensor.matmul(
                    warm_ps[:, 0, :], dummy[:, :128], dummy[:, 128:640],
                    start=True, stop=True,
                )

            # Input DMAs on BOTH HWDGE rings (SP + ACT) — triggers cost
            # ~650ns each and serialize per ring; the two rings share HBM
            # bandwidth, so the first-needed bytes are split across both.
            # Later xt blocks go LAST: with a 1-slot xt pool their triggers
            # carry WAR waits, which would head-of-line-block the ring.
            hk = KT1 // 2
            ch00, bch0 = blocks[0]
            nc.sync.dma_start(
                out=xts[0][:, :hk, :bch0], in_=xt_d[:, :hk, ch00 : ch00 + bch0]
            )
            nc.scalar.dma_start(
                out=xts[0][:, hk:, :bch0], in_=xt_d[:, hk:, ch00 : ch00 + bch0]
            )
            pr0 = 0
            for ci, cw in enumerate(w1_chunks):
                eng = nc.sync if ci % 2 == 0 else nc.scalar
                eng.dma_start(
                    out=w1[:, pr0 : pr0 + cw],
                    in_=w1_d[:, 2 * pr0 : 2 * (pr0 + cw)],
                )
                pr0 += cw
            for g in range(MT2 // 4):
                eng = nc.sync if g % 2 == 0 else nc.scalar
                eng.dma_start(
                    out=w2[:, g * 4 : (g + 1) * 4], in_=w2_d[:, g * 4 : (g + 1) * 4]
                )
            for bi, (ch0, bch) in enumerate(blocks[1:], start=1):
                eng = nc.sync if bi % 2 == 1 else nc.scalar
                eng.dma_start(
                    out=xts[bi][:, :, :bch], in_=xt_d[:, :, ch0 : ch0 + bch]
                )

            for bi, (ch0, bch) in enumerate(blocks):
                xt = xts[bi]
                # ---- FC1 (proj/gate pair per iteration) + SwiGLU ----
                act = apool.tile([P, KT2, 2, CH], BF16, tag="act", name=f"act{bi}")
                for mt in range(MT1H):
                    ps_p = pspool.tile(
                        [P, 2, 512], F32, tag="ps", name=f"psp{bi}_{mt}"
                    )
                    ps_g = pspool.tile(
                        [P, 2, 512], F32, tag="ps", name=f"psg{bi}_{mt}"
                    )
                    for ps, pg in ((ps_p, 0), (ps_g, 1)):
                        for kt in range(KT1):
                            for j in range(bch):
                                nc.tensor.matmul(
                                    ps[:, j, :CH],
                                    w1[:, mt, pg, kt, :],
                                    xt[:, kt, j, :],
                                    start=(kt == 0),
                                    stop=(kt == KT1 - 1),
                                )
                    # SwiGLU: silu on ACT, multiply on DVE (Bacc's
                    # generate_event_semaphores legalizes multi-wait insts).
                    tmp = tpool.tile([P, 2, CH], F32, tag="tmp", name=f"tmp{bi}_{mt}")
                    for j in range(bch):
                        nc.scalar.activation(
                            tmp[:, j], ps_p[:, j, :CH],
                            mybir.ActivationFunctionType.Silu,
                        )
                        nc.vector.tensor_mul(act[:, mt, j], tmp[:, j], ps_g[:, j, :CH])

                # ---- FC2 ----
                for m2 in range(MT2):
                    ps_o = pspool.tile(
                        [P, 2, 512], F32, tag="ps", name=f"pso{bi}_{m2}"
                    )
                    for kt2 in range(KT2):
                        for j in range(bch):
                            nc.tensor.matmul(
                                ps_o[:, j, :CH],
                                w2[:, m2, kt2, :],
                                act[:, kt2, j, :],
                                start=(kt2 == 0),
                                stop=(kt2 == KT2 - 1),
                            )
                    o_sb = opool.tile([P, 2, CH], BF16, tag="o", name=f"osb{bi}_{m2}")
                    nc.scalar.copy(o_sb[:, :bch], ps_o[:, :bch, :CH])
                    eng = nc.sync if m2 % 2 == 0 else nc.scalar
                    eng.dma_start(
                        out=out_d[m2, ch0 : ch0 + bch].rearrange("b p c -> p b c"),
                        in_=o_sb[:, :bch],
                    )

    nc.compile()
    return nc


def _get_graph(NCH: int, CH: int) -> bass.Bass:
    key = (NCH, CH)
    if key not in _graph_cache:
        _graph_cache[key] = _build(NCH, CH)
    return _graph_cache[key]


def _route(router_logits: np.ndarray):
    """Top-2 + softmax, exactly matching jax.lax.top_k tie-breaking."""
    idx = np.argsort(-router_logits, axis=-1, kind="stable")[:, :TOPK]
    tl = np.take_along_axis(router_logits, idx, axis=-1)
    ex = np.exp(tl - tl.max(-1, keepdims=True))
    sc = (ex / ex.sum(-1, keepdims=True)).astype(np.float32)
    return idx, sc


def run(hidden_states, router_logits, w1, w2, trace=False, trace_kwargs=None):
    hs = np.asarray(hidden_states, dtype=np.float32)
    rl = np.asarray(router_logits, dtype=np.float32)
    w1 = np.asarray(w1, dtype=np.float32)
    w2 = np.asarray(w2, dtype=np.float32)
    N, D = hs.shape

    idx, sc = _route(rl)

    tok_lists = []
    for e in range(NUM_EXPERTS):
        toks, slots = np.nonzero(idx == e)
        tok_lists.append((toks, slots))
    cmax = max(len(t) for t, _ in tok_lists)

    NCH = max(1, -(-cmax // 512))
    CH = -(-cmax // (NCH * 2)) * 2  # chunk width, multiple of 2
    C_pad = CH * NCH

    in_maps = []
    for e in range(NUM_EXPERTS):
        toks, _ = tok_lists[e]
        x = np.zeros((C_pad, D), np.float32)
        x[: len(toks)] = hs[toks]
        xt = x.T.reshape(KT1, P, NCH, CH).transpose(1, 0, 2, 3).astype(np_bf16)
        # [p, mt, kt, m] with the mt axis pair-interleaved: proj tile mt and
        # gate tile mt+MT1H land adjacently so pair mt needs one DMA chunk.
        w1e = w1[e].reshape(KT1, P, MT1, P).transpose(1, 2, 0, 3)[:, _W1_ORDER]
        w1e = w1e.astype(np_bf16)
        w2e = w2[e].reshape(KT2, P, MT2, P).transpose(1, 2, 0, 3).astype(np_bf16)
        in_maps.append({"xt": xt, "w1": w1e, "w2": w2e})

    nc = _get_graph(NCH, CH)

    res = None
    for attempt in range(4):
        try:
            res = run_bass_kernel_spmd(
                nc,
                in_maps,
                core_ids=list(range(NCORES)),
                trace=trace,
                **(trace_kwargs or {}),
            )
            break
        except Exception:
            if attempt == 3:
                raise
            time.sleep(15 * (attempt + 1))

    out = np.zeros((N, D), np.float32)
    for e in range(NUM_EXPERTS):
        toks, slots = tok_lists[e]
        oT = np.asarray(res.results[e]["out"]).astype(np.float32)
        oT = oT.transpose(0, 2, 1, 3).reshape(HIDDEN, C_pad)
        out[toks] += sc[toks, slots][:, None] * oT[:, : len(toks)].T
    return out, res


def kernel(hidden_states, router_logits, w1, w2):
    out, _ = run(hidden_states, router_logits, w1, w2)
    return out
